# revision 5
# baseline (speedup 1.0000x reference)
"""Trainium2 Bass kernel for the CAM factorized-attention module.

Reference computation (per batch element b, C=256, N=P*H*W=12288, h=8 heads,
Ch=32):
    x1   = x[b].reshape(C, N).T                      # [N, C]
    qkv  = x1 @ W_qkv + b_qkv                        # [N, 3C]
    q, k, v  (each [h, N, Ch])
    kw   = softmax(k, axis=N)
    kv   = kw^T @ v (per head)                       # [h, Ch, Ch]
    fa   = q @ kv                                    # [h, N, Ch]
    out  = (scale * fa).reshape(N, C) @ W_proj + b_proj
    res  = gamma * out.T.reshape(C, P, H, W) + x[b]

Sharding: data-parallel over B — core i computes batch element i. No
collectives. All matmuls run in bf16 with fp32 PSUM accumulation; the
attention branch is ~0.3% of the output magnitude (output = x + gamma*attn),
so bf16 rounding contributes ~5e-6 relative error end to end.

Host-side algebraic folds (exact):
  * k bias cancels in softmax (constant along the softmax axis)  -> dropped.
  * v bias:   kv_true = (E^T v_raw)/S + b_v (row vec)            -> tiny add.
  * softmax:  no max-subtraction needed (|k| < ~4); normalization is applied
              to the tiny [Ch, Ch] kv matrix instead of the [N, C] weights.
  * scale & gamma fold into W_proj;  gamma folds into b_proj.
  * q bias applied per-partition during the qT PSUM->SBUF copy.

On-chip layouts (per core):
  xbf   [2][128, 12288] bf16   rows = channel c, cols = token n   (resident)
  qT    [2][128, 12288] bf16   rows = (head, ch) of q^T           (resident)
  k||v  per 128-token chunk: PSUM [128, 512] (tokens on partitions)
  E     exp(k) bf16 [128, 256];  vb = [v | ones] bf16 [128, 257]
  kvps  PSUM [128, 257] accumulates E^T @ [v|1] over all 96 chunks:
        per-head diagonal 32x32 blocks + column 256 = softmax denominators
  kvblk [2][128, 128] bf16: block-diagonal per-head kv (zeros elsewhere) so
        fa for 4 heads is ONE full-array matmul (off-diag blocks contribute 0)
"""

import sys

sys.path.insert(0, "/opt/trn_rl_repo")

import numpy as np
import ml_dtypes

import concourse.bass as bass
import concourse.bacc as bacc
import concourse.mybir as mybir
from concourse.tile import TileContext
from concourse.bass_utils import run_bass_kernel_spmd

FP32 = mybir.dt.float32
BF16 = mybir.dt.bfloat16
AF = mybir.ActivationFunctionType
ALU = None  # loaded lazily with alu_op_type

C = 256
N = 12288
H = 8
CH = 32
NCORES = 8
NCHUNK = N // 128  # 96 chunks of 128 tokens
NJUMBO = N // 512  # 24 chunks of 512 tokens

_CACHE = {}


def _build_nc():
    from concourse.alu_op_type import AluOpType

    nc = bacc.Bacc(trn_type="TRN2", target_bir_lowering=False)

    xbf_d = nc.declare_dram_parameter("xbf", [2, 128, N], BF16, False)
    xf_d = nc.declare_dram_parameter("xf", [2, 128, N], FP32, False)
    wq_d = nc.declare_dram_parameter("wq", [2, 128, 256], BF16, False)
    wkv_d = nc.declare_dram_parameter("wkv", [2, 128, 512], BF16, False)
    wp_d = nc.declare_dram_parameter("wp", [2, 128, 256], BF16, False)
    bq_d = nc.declare_dram_parameter("bq", [2, 128, 1], FP32, False)
    bp_d = nc.declare_dram_parameter("bp", [2, 128, 1], FP32, False)
    bv_d = nc.declare_dram_parameter("bv", [2, 128, 32], FP32, False)
    out_d = nc.declare_dram_parameter("out", [2, 128, N], FP32, True)

    with TileContext(nc) as tc:
        with (
            tc.tile_pool(name="const", bufs=1) as const,
            tc.tile_pool(name="resident", bufs=1) as resident,
        ):
            # --- resident tensors -------------------------------------------
            xbf = [resident.tile([128, N], BF16, name=f"xbf{t}", tag=f"xbf{t}") for t in range(2)]
            qT = [resident.tile([128, N], BF16, name=f"qT{t}", tag=f"qT{t}") for t in range(2)]
            wq = [const.tile([128, 256], BF16, name=f"wq{t}", tag=f"wq{t}") for t in range(2)]
            wkv = [const.tile([128, 512], BF16, name=f"wkv{t}", tag=f"wkv{t}") for t in range(2)]
            wp = [const.tile([128, 256], BF16, name=f"wp{t}", tag=f"wp{t}") for t in range(2)]
            bq = [const.tile([128, 1], FP32, name=f"bq{t}", tag=f"bq{t}") for t in range(2)]
            bp = [const.tile([128, 1], FP32, name=f"bp{t}", tag=f"bp{t}") for t in range(2)]
            bv = [const.tile([128, 32], FP32, name=f"bv{t}", tag=f"bv{t}") for t in range(2)]
            kvblk = [const.tile([128, 128], BF16, name=f"kvblk{t}", tag=f"kvblk{t}") for t in range(2)]
            vb = [const.tile([128, 257], BF16, name=f"vb{j}", tag=f"vb{j}") for j in range(3)]
            recip = [const.tile([128, 1], FP32, name=f"recip{t}", tag=f"recip{t}") for t in range(2)]

            for t in range(2):
                nc.sync.dma_start(xbf[t][:], xbf_d[t])
                nc.sync.dma_start(wq[t][:], wq_d[t])
                nc.sync.dma_start(wkv[t][:], wkv_d[t])
                nc.sync.dma_start(wp[t][:], wp_d[t])
                nc.sync.dma_start(bq[t][:], bq_d[t])
                nc.sync.dma_start(bp[t][:], bp_d[t])
                nc.sync.dma_start(bv[t][:], bv_d[t])
                nc.vector.memset(kvblk[t][:], 0.0)
            for j in range(3):
                nc.vector.memset(vb[j][:, 256:257], 1.0)

            # --- phase 1: qkv, exp, kv accumulation -------------------------
            with (
                tc.tile_pool(name="p1ps", bufs=1, space="PSUM") as p1ps,
                tc.tile_pool(name="kvp_ps", bufs=3, space="PSUM") as kvp_ps,
                tc.tile_pool(name="qps_ps", bufs=2, space="PSUM") as qps_ps,
                tc.tile_pool(name="ework", bufs=3) as ework,
            ):
                kvps = [p1ps.tile([128, 257], FP32, name=f"kvps{t}", tag=f"kvps{t}") for t in range(2)]

                for ci in range(NCHUNK):
                    n0 = ci * 128
                    first, last = ci == 0, ci == NCHUNK - 1

                    kvp = kvp_ps.tile([128, 512], FP32, name="kvp", tag="kvp")
                    nc.tensor.matmul(
                        kvp[:], lhsT=xbf[0][:, n0 : n0 + 128], rhs=wkv[0][:],
                        start=True, stop=False,
                    )
                    nc.tensor.matmul(
                        kvp[:], lhsT=xbf[1][:, n0 : n0 + 128], rhs=wkv[1][:],
                        start=False, stop=True,
                    )
                    E = ework.tile([128, 256], BF16, name="E", tag="E")
                    nc.scalar.activation(E[:], kvp[:, 0:256], AF.Exp)
                    v = vb[ci % 3]
                    nc.vector.tensor_copy(v[:, 0:256], kvp[:, 256:512])

                    nc.tensor.matmul(
                        kvps[0][:], lhsT=E[:, 0:128], rhs=v[:],
                        start=first, stop=last, skip_group_check=True,
                    )
                    nc.tensor.matmul(
                        kvps[1][:], lhsT=E[:, 128:256], rhs=v[:],
                        start=first, stop=last, skip_group_check=True,
                    )

                    if ci % 4 == 0:
                        m0 = n0
                        for mt in range(2):
                            qp = qps_ps.tile([128, 512], FP32, name="qp", tag="qp")
                            nc.tensor.matmul(
                                qp[:], lhsT=wq[0][:, mt * 128 : mt * 128 + 128],
                                rhs=xbf[0][:, m0 : m0 + 512], start=True, stop=False,
                            )
                            nc.tensor.matmul(
                                qp[:], lhsT=wq[1][:, mt * 128 : mt * 128 + 128],
                                rhs=xbf[1][:, m0 : m0 + 512], start=False, stop=True,
                            )
                            nc.vector.tensor_scalar_add(
                                qT[mt][:, m0 : m0 + 512], qp[:], bq[mt][:]
                            )

                # --- finalize kv: normalize rows, add v bias ----------------
                for t in range(2):
                    nc.vector.reciprocal(recip[t][:], kvps[t][:, 256:257])
                    for g in range(4):
                        r0 = g * 32
                        nc.vector.scalar_tensor_tensor(
                            kvblk[t][r0 : r0 + 32, r0 : r0 + 32],
                            kvps[t][r0 : r0 + 32, r0 : r0 + 32],
                            recip[t][r0 : r0 + 32, :],
                            bv[t][r0 : r0 + 32, :],
                            op0=AluOpType.mult,
                            op1=AluOpType.add,
                        )

            # --- phase 2: fa = kv^T @ qT, proj, bias + residual -------------
            with (
                tc.tile_pool(name="fa_ps", bufs=4, space="PSUM") as fa_ps,
                tc.tile_pool(name="pp_ps", bufs=4, space="PSUM") as pp_ps,
                tc.tile_pool(name="p2work", bufs=4) as p2work,
                tc.tile_pool(name="p2out", bufs=3) as p2out,
            ):
                for cj in range(NJUMBO):
                    n0 = cj * 512
                    fsb = []
                    for t in range(2):
                        fap = fa_ps.tile([128, 512], FP32, name="fap", tag="fap")
                        nc.tensor.matmul(
                            fap[:], lhsT=kvblk[t][:], rhs=qT[t][:, n0 : n0 + 512],
                            start=True, stop=True,
                        )
                        f = p2work.tile([128, 512], BF16, name="fsb", tag="fsb")
                        nc.scalar.copy(f[:], fap[:])
                        fsb.append(f)
                    for mt in range(2):
                        pp = pp_ps.tile([128, 512], FP32, name="pp", tag="pp")
                        nc.tensor.matmul(
                            pp[:], lhsT=wp[0][:, mt * 128 : mt * 128 + 128],
                            rhs=fsb[0][:], start=True, stop=False,
                        )
                        nc.tensor.matmul(
                            pp[:], lhsT=wp[1][:, mt * 128 : mt * 128 + 128],
                            rhs=fsb[1][:], start=False, stop=True,
                        )
                        xin = p2out.tile([128, 512], FP32, name="xin", tag="xin")
                        nc.sync.dma_start(xin[:], xf_d[mt, :, n0 : n0 + 512])
                        osb = p2out.tile([128, 512], FP32, name="osb", tag="osb")
                        nc.vector.scalar_tensor_tensor(
                            osb[:], pp[:], bp[mt][:], xin[:],
                            op0=AluOpType.add, op1=AluOpType.add,
                        )
                        nc.sync.dma_start(out_d[mt, :, n0 : n0 + 512], osb[:])
    nc.finalize()
    return nc


def _get_nc():
    if "nc" not in _CACHE:
        _CACHE["nc"] = _build_nc()
    return _CACHE["nc"]


def _prep_in_maps(x, W_qkv, b_qkv, W_proj, b_proj, gamma):
    bf = ml_dtypes.bfloat16
    scale = CH ** (-0.5)
    g = float(np.asarray(gamma).reshape(-1)[0])

    Wq = np.ascontiguousarray(
        W_qkv[:, 0:256].reshape(2, 128, 256)).astype(bf)
    Wkv = np.ascontiguousarray(
        W_qkv[:, 256:768].reshape(2, 128, 512)).astype(bf)
    Wp = np.ascontiguousarray(
        (W_proj * (scale * g)).reshape(2, 128, 256)).astype(bf)
    bq = np.ascontiguousarray(
        b_qkv[0:256].reshape(2, 128, 1)).astype(np.float32)
    bp = np.ascontiguousarray(
        (g * b_proj).reshape(2, 128, 1)).astype(np.float32)
    # bv[t][p, cv] = b_qkv[512 + (t*4 + p//32)*32 + cv]
    bv = np.ascontiguousarray(
        np.broadcast_to(
            b_qkv[512:768].reshape(2, 4, 1, 32), (2, 4, 32, 32)
        ).reshape(2, 128, 32)
    ).astype(np.float32)

    in_maps = []
    for b in range(NCORES):
        xb = np.ascontiguousarray(x[b].reshape(C, N))
        in_maps.append(
            {
                "xbf": xb.reshape(2, 128, N).astype(bf),
                "xf": xb.reshape(2, 128, N),
                "wq": Wq, "wkv": Wkv, "wp": Wp,
                "bq": bq, "bp": bp, "bv": bv,
            }
        )
    return in_maps


def kernel(x, W_qkv, b_qkv, W_proj, b_proj, gamma, _trace=False, _trace_kwargs=None):
    x = np.asarray(x, dtype=np.float32)
    nc = _get_nc()
    in_maps = _prep_in_maps(
        x,
        np.asarray(W_qkv, np.float32),
        np.asarray(b_qkv, np.float32),
        np.asarray(W_proj, np.float32),
        np.asarray(b_proj, np.float32),
        np.asarray(gamma, np.float32),
    )
    kw = {}
    if _trace:
        kw = {"trace": True, **(_trace_kwargs or {})}
    res = run_bass_kernel_spmd(nc, in_maps, list(range(NCORES)), **kw)
    out = np.stack(
        [res.results[b]["out"].reshape(C, 3, 64, 64) for b in range(NCORES)]
    ).astype(np.float32)
    if _trace:
        return out, res
    return out


# revision 13
# speedup vs baseline: 1.1881x; 1.1881x over previous
"""Trainium2 Bass kernel for the CAM factorized-attention module.

Reference computation (per batch element b, C=256, N=P*H*W=12288, h=8 heads,
Ch=32):
    x1   = x[b].reshape(C, N).T                      # [N, C]
    qkv  = x1 @ W_qkv + b_qkv                        # [N, 3C]
    q, k, v  (each [h, N, Ch])
    kw   = softmax(k, axis=N)
    kv   = kw^T @ v (per head)                       # [h, Ch, Ch]
    fa   = q @ kv                                    # [h, N, Ch]
    out  = (scale * fa).reshape(N, C) @ W_proj + b_proj
    res  = gamma * out.T.reshape(C, P, H, W) + x[b]

Sharding: data-parallel over B — core i computes batch element i. No
collectives. All matmuls run in bf16 with fp32 PSUM accumulation; the
attention branch is ~0.3% of the output magnitude (output = x + gamma*attn),
so bf16 rounding contributes ~5e-6 relative error end to end.

Host-side algebraic folds (exact):
  * k bias cancels in softmax (constant along the softmax axis)  -> dropped.
  * v bias:   kv_true = (E^T v_raw)/S + b_v (row vec)            -> tiny add.
  * softmax:  no max-subtraction needed (|k| < ~4); normalization is applied
              to the tiny [Ch, Ch] kv matrix instead of the [N, C] weights.
  * scale & gamma fold into W_proj;  gamma folds into b_proj.
  * q bias applied per-partition during the qT PSUM->SBUF copy.

On-chip layouts (per core):
  xbf   [2][128, 12288] bf16   rows = channel c, cols = token n   (resident)
  qT    [2][128, 12288] bf16   rows = (head, ch) of q^T           (resident)
  k||v  per 128-token chunk: PSUM [128, 512] (tokens on partitions)
  E     exp(k) bf16 [128, 256];  vb = [v | ones] bf16 [128, 257]
  kvps  PSUM [128, 257] accumulates E^T @ [v|1] over all 96 chunks:
        per-head diagonal 32x32 blocks + column 256 = softmax denominators
  kvblk [2][128, 128] bf16: block-diagonal per-head kv (zeros elsewhere) so
        fa for 4 heads is ONE full-array matmul (off-diag blocks contribute 0)
"""

import sys

sys.path.insert(0, "/opt/trn_rl_repo")

import numpy as np
import ml_dtypes

import concourse.bass as bass
import concourse.bacc as bacc
import concourse.mybir as mybir
from concourse.tile import TileContext
from concourse.bass_utils import run_bass_kernel_spmd

FP32 = mybir.dt.float32
BF16 = mybir.dt.bfloat16
AF = mybir.ActivationFunctionType
ALU = None  # loaded lazily with alu_op_type

C = 256
N = 12288
H = 8
CH = 32
NCORES = 8
NCHUNK = N // 128  # 96 chunks of 128 tokens
NJUMBO = N // 512  # 24 chunks of 512 tokens

_CACHE = {}


def _build_nc(debug=False):
    from concourse.alu_op_type import AluOpType

    nc = bacc.Bacc(trn_type="TRN2", target_bir_lowering=False)

    xbf_d = nc.declare_dram_parameter("xbf", [2, 128, N], BF16, False)
    xf_d = nc.declare_dram_parameter("xf", [2, 128, N], FP32, False)
    wq_d = nc.declare_dram_parameter("wq", [2, 128, 256], BF16, False)
    wkv_d = nc.declare_dram_parameter("wkv", [2, 128, 512], BF16, False)
    wp_d = nc.declare_dram_parameter("wp", [2, 128, 256], BF16, False)
    bq_d = nc.declare_dram_parameter("bq", [2, 128, 1], FP32, False)
    bp_d = nc.declare_dram_parameter("bp", [2, 128, 1], FP32, False)
    bv_d = nc.declare_dram_parameter("bv", [2, 128, 32], FP32, False)
    out_d = nc.declare_dram_parameter("out", [2, 128, N], FP32, True)
    if debug:
        dbg_qT = nc.declare_dram_parameter("dbg_qT", [2, 128, N], BF16, True)
        dbg_E = nc.declare_dram_parameter("dbg_E", [128, 256], BF16, True)
        dbg_v = nc.declare_dram_parameter("dbg_v", [128, 257], BF16, True)
        dbg_kvps = nc.declare_dram_parameter("dbg_kvps", [2, 128, 257], FP32, True)
        dbg_kvblk = nc.declare_dram_parameter("dbg_kvblk", [2, 128, 128], BF16, True)
        dbg_fsb = nc.declare_dram_parameter("dbg_fsb", [2, 128, 512], BF16, True)

    with TileContext(nc) as tc:
        with (
            tc.tile_pool(name="const", bufs=1) as const,
            tc.tile_pool(name="resident", bufs=1) as resident,
        ):
            # --- resident tensors -------------------------------------------
            xbf = [resident.tile([128, N], BF16, name=f"xbf{t}", tag=f"xbf{t}") for t in range(2)]
            qT = [resident.tile([128, N], BF16, name=f"qT{t}", tag=f"qT{t}") for t in range(2)]
            wq = [const.tile([128, 256], BF16, name=f"wq{t}", tag=f"wq{t}") for t in range(2)]
            wkv = [const.tile([128, 512], BF16, name=f"wkv{t}", tag=f"wkv{t}") for t in range(2)]
            wp = [const.tile([128, 256], BF16, name=f"wp{t}", tag=f"wp{t}") for t in range(2)]
            bq = [const.tile([128, 1], FP32, name=f"bq{t}", tag=f"bq{t}") for t in range(2)]
            bp = [const.tile([128, 1], FP32, name=f"bp{t}", tag=f"bp{t}") for t in range(2)]
            bv = [const.tile([128, 32], FP32, name=f"bv{t}", tag=f"bv{t}") for t in range(2)]
            kvblk = [const.tile([128, 128], BF16, name=f"kvblk{t}", tag=f"kvblk{t}") for t in range(2)]
            vb = [const.tile([128, 257], BF16, name=f"vb{j}", tag=f"vb{j}") for j in range(3)]
            recip = [const.tile([128, 1], FP32, name=f"recip{t}", tag=f"recip{t}") for t in range(2)]

            for t in range(2):
                nc.sync.dma_start(xbf[t][:], xbf_d[t])
                nc.sync.dma_start(wq[t][:], wq_d[t])
                nc.sync.dma_start(wkv[t][:], wkv_d[t])
                nc.sync.dma_start(wp[t][:], wp_d[t])
                nc.sync.dma_start(bq[t][:], bq_d[t])
                nc.sync.dma_start(bp[t][:], bp_d[t])
                nc.sync.dma_start(bv[t][:], bv_d[t])
                nc.vector.memset(kvblk[t][:], 0.0)
            for j in range(3):
                nc.vector.memset(vb[j][:, 256:257], 1.0)

            # --- phase 1: qkv, exp, kv accumulation -------------------------
            with (
                tc.tile_pool(name="p1ps", bufs=1, space="PSUM") as p1ps,
                tc.tile_pool(name="kvp_ps", bufs=3, space="PSUM") as kvp_ps,
                tc.tile_pool(name="qps_ps", bufs=2, space="PSUM") as qps_ps,
                tc.tile_pool(name="ework", bufs=3) as ework,
            ):
                kvps = [p1ps.tile([128, 257], FP32, name=f"kvps{t}", tag=f"kvps{t}") for t in range(2)]

                for ci in range(NCHUNK):
                    n0 = ci * 128
                    first, last = ci == 0, ci == NCHUNK - 1

                    kvp = kvp_ps.tile([128, 512], FP32, name="kvp", tag="kvp")
                    nc.tensor.matmul(
                        kvp[:], lhsT=xbf[0][:, n0 : n0 + 128], rhs=wkv[0][:],
                        start=True, stop=False,
                    )
                    nc.tensor.matmul(
                        kvp[:], lhsT=xbf[1][:, n0 : n0 + 128], rhs=wkv[1][:],
                        start=False, stop=True,
                    )
                    E = ework.tile([128, 256], BF16, name="E", tag="E")
                    nc.scalar.activation(E[:], kvp[:, 0:256], AF.Exp)
                    v = vb[ci % 3]
                    nc.vector.tensor_copy(v[:, 0:256], kvp[:, 256:512])
                    if debug and ci == 0:
                        nc.sync.dma_start(dbg_E[:, :], E[:])
                        nc.sync.dma_start(dbg_v[:, :], v[:])

                    nc.tensor.matmul(
                        kvps[0][:], lhsT=E[:, 0:128], rhs=v[:],
                        start=first, stop=last, skip_group_check=True,
                    )
                    nc.tensor.matmul(
                        kvps[1][:], lhsT=E[:, 128:256], rhs=v[:],
                        start=first, stop=last, skip_group_check=True,
                    )

                    if ci % 4 == 0:
                        m0 = n0
                        for mt in range(2):
                            qp = qps_ps.tile([128, 512], FP32, name="qp", tag="qp")
                            nc.tensor.matmul(
                                qp[:], lhsT=wq[0][:, mt * 128 : mt * 128 + 128],
                                rhs=xbf[0][:, m0 : m0 + 512], start=True, stop=False,
                            )
                            nc.tensor.matmul(
                                qp[:], lhsT=wq[1][:, mt * 128 : mt * 128 + 128],
                                rhs=xbf[1][:, m0 : m0 + 512], start=False, stop=True,
                            )
                            nc.vector.tensor_scalar_add(
                                qT[mt][:, m0 : m0 + 512], qp[:], bq[mt][:]
                            )

                # --- finalize kv: normalize rows, add v bias ----------------
                if debug:
                    for t in range(2):
                        kvcp = ework.tile(
                            [128, 257], FP32, name=f"kvcp{t}", tag=f"kvcp{t}"
                        )
                        nc.vector.tensor_copy(kvcp[:], kvps[t][:])
                        nc.sync.dma_start(dbg_kvps[t], kvcp[:])
                for t in range(2):
                    nc.vector.reciprocal(recip[t][:], kvps[t][:, 256:257])
                    for g in range(4):
                        r0 = g * 32
                        c0 = (t * 4 + g) * 32  # v columns are global 0..255
                        nc.vector.scalar_tensor_tensor(
                            kvblk[t][r0 : r0 + 32, r0 : r0 + 32],
                            kvps[t][r0 : r0 + 32, c0 : c0 + 32],
                            recip[t][r0 : r0 + 32, :],
                            bv[t][r0 : r0 + 32, :],
                            op0=AluOpType.mult,
                            op1=AluOpType.add,
                        )

            # --- phase 2: fa = kv^T @ qT, proj, bias + residual -------------
            with (
                tc.tile_pool(name="fa_ps", bufs=4, space="PSUM") as fa_ps,
                tc.tile_pool(name="pp_ps", bufs=4, space="PSUM") as pp_ps,
                tc.tile_pool(name="p2work", bufs=4) as p2work,
                tc.tile_pool(name="p2out", bufs=3) as p2out,
            ):
                for cj in range(NJUMBO):
                    n0 = cj * 512
                    fsb = []
                    for t in range(2):
                        fap = fa_ps.tile([128, 512], FP32, name="fap", tag="fap")
                        nc.tensor.matmul(
                            fap[:], lhsT=kvblk[t][:], rhs=qT[t][:, n0 : n0 + 512],
                            start=True, stop=True,
                        )
                        f = p2work.tile([128, 512], BF16, name="fsb", tag="fsb")
                        nc.scalar.copy(f[:], fap[:])
                        fsb.append(f)
                        if debug and cj == 0:
                            nc.sync.dma_start(dbg_fsb[t], f[:])
                    for mt in range(2):
                        pp = pp_ps.tile([128, 512], FP32, name="pp", tag="pp")
                        nc.tensor.matmul(
                            pp[:], lhsT=wp[0][:, mt * 128 : mt * 128 + 128],
                            rhs=fsb[0][:], start=True, stop=False,
                        )
                        nc.tensor.matmul(
                            pp[:], lhsT=wp[1][:, mt * 128 : mt * 128 + 128],
                            rhs=fsb[1][:], start=False, stop=True,
                        )
                        xin = p2out.tile([128, 512], FP32, name="xin", tag="xin")
                        nc.sync.dma_start(xin[:], xf_d[mt, :, n0 : n0 + 512])
                        osb = p2out.tile([128, 512], FP32, name="osb", tag="osb")
                        nc.vector.scalar_tensor_tensor(
                            osb[:], pp[:], bp[mt][:], xin[:],
                            op0=AluOpType.add, op1=AluOpType.add,
                        )
                        nc.sync.dma_start(out_d[mt, :, n0 : n0 + 512], osb[:])
            if debug:
                for t in range(2):
                    nc.sync.dma_start(dbg_qT[t], qT[t][:])
                    nc.sync.dma_start(dbg_kvblk[t], kvblk[t][:])
    nc.finalize()
    return nc


def _get_nc():
    if "nc" not in _CACHE:
        _CACHE["nc"] = _build_nc()
    return _CACHE["nc"]


def _prep_in_maps(x, W_qkv, b_qkv, W_proj, b_proj, gamma):
    bf = ml_dtypes.bfloat16
    scale = CH ** (-0.5)
    g = float(np.asarray(gamma).reshape(-1)[0])

    Wq = np.ascontiguousarray(
        W_qkv[:, 0:256].reshape(2, 128, 256)).astype(bf)
    Wkv = np.ascontiguousarray(
        W_qkv[:, 256:768].reshape(2, 128, 512)).astype(bf)
    Wp = np.ascontiguousarray(
        (W_proj * (scale * g)).reshape(2, 128, 256)).astype(bf)
    bq = np.ascontiguousarray(
        b_qkv[0:256].reshape(2, 128, 1)).astype(np.float32)
    bp = np.ascontiguousarray(
        (g * b_proj).reshape(2, 128, 1)).astype(np.float32)
    # bv[t][p, cv] = b_qkv[512 + (t*4 + p//32)*32 + cv]
    bv = np.ascontiguousarray(
        np.broadcast_to(
            b_qkv[512:768].reshape(2, 4, 1, 32), (2, 4, 32, 32)
        ).reshape(2, 128, 32)
    ).astype(np.float32)

    in_maps = []
    for b in range(NCORES):
        xb = np.ascontiguousarray(x[b].reshape(C, N))
        in_maps.append(
            {
                "xbf": xb.reshape(2, 128, N).astype(bf),
                "xf": xb.reshape(2, 128, N),
                "wq": Wq, "wkv": Wkv, "wp": Wp,
                "bq": bq, "bp": bp, "bv": bv,
            }
        )
    return in_maps


def kernel(x, W_qkv, b_qkv, W_proj, b_proj, gamma, _trace=False, _trace_kwargs=None):
    x = np.asarray(x, dtype=np.float32)
    nc = _get_nc()
    in_maps = _prep_in_maps(
        x,
        np.asarray(W_qkv, np.float32),
        np.asarray(b_qkv, np.float32),
        np.asarray(W_proj, np.float32),
        np.asarray(b_proj, np.float32),
        np.asarray(gamma, np.float32),
    )
    kw = {}
    if _trace:
        kw = {"trace": True, **(_trace_kwargs or {})}
    res = run_bass_kernel_spmd(nc, in_maps, list(range(NCORES)), **kw)
    out = np.stack(
        [res.results[b]["out"].reshape(C, 3, 64, 64) for b in range(NCORES)]
    ).astype(np.float32)
    if _trace:
        return out, res
    return out
